# revision 2
# baseline (speedup 1.0000x reference)
"""Packed-sequence GRU (single layer) on 8 Trainium2 NeuronCores.

Strategy (data-parallel over batch, 32 rows/core):
  - Host pre-transposes x to x^T [E, T*Bl] and converts to bf16, so the
    input-projection matmul Gi = W_ih @ x_t needs no on-device transpose.
  - Gi lives in PSUM in 8-step blocks; biases and the packed-sequence
    length mask are folded in with K=2 matmuls (row0 = combined bias,
    row1 = +30.0 on the z-gate rows of inactive (t, b) entries, which
    saturates z -> 1 so frozen rows keep their hidden state exactly like
    the reference's masking, to ~1e-13).
  - The recurrent matmul gh = W_hh @ h accumulates directly onto the Gi
    PSUM regions for the r/z gates (the PE does the gi+gh add for free).
    The n gate keeps gi/gh separate (GRU applies r only to the gh part).
  - Gates run in [feature-partition, batch-free] layout: sigmoid/tanh on
    ScalarE, the rest on VectorE, with b_hn folded into one fused
    scalar_tensor_tensor op per H-chunk.
  - h is kept in fp32 (master) and mirrored to bf16 for the next step's
    matmul.
"""

import sys

for _p in ("/opt/trn_rl_repo", "/root/.axon_site/_ro/trn_rl_repo"):
    if _p not in sys.path:
        sys.path.insert(0, _p)

import numpy as np
import ml_dtypes

import concourse.bacc as bacc
import concourse.tile as tile
from concourse import mybir
from concourse.bass_utils import run_bass_kernel_spmd
from concourse.alu_op_type import AluOpType

BF16 = ml_dtypes.bfloat16
N_CORES = 8

_PROGRAM_CACHE = {}
LAST_RESULT = None  # stashed BassKernelResults for test harness introspection


def _build_program(T, Bl, E, H, S=8):
    """Emit the per-core GRU program. Same program runs SPMD on all cores."""
    assert E == 256 and H == 256 and Bl == 32 and T % S == 0
    TB = T * Bl
    NB = TB // (S * Bl)  # number of 8-step blocks
    G3 = 3 * H  # 768
    bf = mybir.dt.bfloat16
    f32 = mybir.dt.float32

    nc = bacc.Bacc(None, target_bir_lowering=False)
    d_xT = nc.declare_dram_parameter("xT", [E, TB], bf, False)
    d_wih = nc.declare_dram_parameter("wihT", [E, G3], bf, False)
    d_whh = nc.declare_dram_parameter("whhT", [H, G3], bf, False)
    d_bias = nc.declare_dram_parameter("biasL", [2, G3], bf, False)
    d_bmr = nc.declare_dram_parameter("bmr", [2, TB], bf, False)
    d_bhn = nc.declare_dram_parameter("bhn", [128, 2], f32, False)
    d_hT = nc.declare_dram_parameter("hT", [2 * 128, Bl], f32, True)

    with tile.TileContext(nc) as tc:
        with (
            tc.tile_pool(name="consts", bufs=1) as consts,
            tc.tile_pool(name="psum", bufs=2, space="PSUM") as psum,
            tc.tile_pool(name="gates", bufs=3) as gates,
        ):
            # --- constants / state ---
            xsb = consts.tile([128, 2, TB], bf)
            wih_sb = consts.tile([128, 2, G3], bf)
            whh_sb = consts.tile([128, 2, G3], bf)
            bias_sb = consts.tile([2, G3], bf)
            bmr_sb = consts.tile([2, TB], bf)
            bhn_sb = consts.tile([128, 2], f32)
            h32 = consts.tile([128, 2, Bl], f32)
            hb = consts.tile([128, 2, Bl], bf)

            for c in (0, 1):
                nc.sync.dma_start(out=xsb[:, c, :], in_=d_xT[c * 128:(c + 1) * 128, :])
                nc.sync.dma_start(out=wih_sb[:, c, :], in_=d_wih[c * 128:(c + 1) * 128, :])
                nc.sync.dma_start(out=whh_sb[:, c, :], in_=d_whh[c * 128:(c + 1) * 128, :])
            nc.sync.dma_start(out=bias_sb[:], in_=d_bias[:])
            nc.sync.dma_start(out=bmr_sb[:], in_=d_bmr[:])
            nc.sync.dma_start(out=bhn_sb[:], in_=d_bhn[:])
            nc.vector.memset(h32[:], 0.0)
            nc.vector.memset(hb[:], 0.0)

            # PSUM block layout (free-dim offsets within a [128, 2048] tile):
            #   rz   : jt*256 + s*32 + b   for jt in 0..3 (r_c0, r_c1, z_c0, z_c1)
            #   n_gi : 1024 + c*256 + s*32 + b
            #   n_gh : 1536 + s*64 + c*32 + b
            NSTEP = S * Bl  # 256 columns per block

            for blk in range(NB):
                pb = psum.tile([128, 2048], f32)
                col0 = blk * NSTEP

                # Gi fill: 12 matmuls, N=256
                for jt in range(6):
                    off = jt * 256 if jt < 4 else 1024 + (jt - 4) * 256
                    for kc in (0, 1):
                        nc.tensor.matmul(
                            pb[:, off:off + NSTEP],
                            lhsT=wih_sb[:, kc, jt * 128:(jt + 1) * 128],
                            rhs=xsb[:, kc, col0:col0 + NSTEP],
                            start=(kc == 0 and jt in (0, 2, 4)),
                            stop=False,
                        )
                # bias + z-saturation mask: K=2 matmuls
                for jt in range(6):
                    off = jt * 256 if jt < 4 else 1024 + (jt - 4) * 256
                    nc.tensor.matmul(
                        pb[:, off:off + NSTEP],
                        lhsT=bias_sb[:, jt * 128:(jt + 1) * 128],
                        rhs=bmr_sb[:, col0:col0 + NSTEP],
                        start=False,
                        stop=False,
                    )

                rz_all = pb[:, 0:1024].rearrange("p (jt s b) -> p jt s b", jt=4, s=S)
                ngi_all = pb[:, 1024:1536].rearrange("p (c s b) -> p c s b", c=2, s=S)

                for s in range(S):
                    # recurrent matmuls accumulate onto Gi (r/z) / write n_gh
                    for jt in range(4):
                        off = jt * 256 + s * 32
                        for kc in (0, 1):
                            nc.tensor.matmul(
                                pb[:, off:off + 32],
                                lhsT=whh_sb[:, kc, jt * 128:(jt + 1) * 128],
                                rhs=hb[:, kc, :],
                                start=False,
                                stop=(kc == 1),
                            )
                    for jt in (4, 5):
                        off = 1536 + s * 64 + (jt - 4) * 32
                        for kc in (0, 1):
                            nc.tensor.matmul(
                                pb[:, off:off + 32],
                                lhsT=whh_sb[:, kc, jt * 128:(jt + 1) * 128],
                                rhs=hb[:, kc, :],
                                start=(blk >= 0 and s == 0 and jt == 4 and kc == 0),
                                stop=(kc == 1),
                            )

                    # gates
                    rz_t = gates.tile([128, 4, Bl], f32)
                    nc.scalar.activation(
                        out=rz_t[:],
                        in_=rz_all[:, :, s, :],
                        func=mybir.ActivationFunctionType.Sigmoid,
                    )
                    p_t = gates.tile([128, 2, Bl], f32)
                    for c in (0, 1):
                        nc.vector.scalar_tensor_tensor(
                            out=p_t[:, c, :],
                            in0=pb[:, 1536 + s * 64 + c * 32: 1536 + s * 64 + c * 32 + 32],
                            scalar=bhn_sb[:, c:c + 1],
                            in1=rz_t[:, c, :],
                            op0=AluOpType.add,
                            op1=AluOpType.mult,
                        )
                    q_t = gates.tile([128, 2, Bl], f32)
                    nc.vector.tensor_tensor(
                        out=q_t[:], in0=p_t[:], in1=ngi_all[:, :, s, :], op=AluOpType.add
                    )
                    n_t = gates.tile([128, 2, Bl], f32)
                    nc.scalar.activation(
                        out=n_t[:], in_=q_t[:],
                        func=mybir.ActivationFunctionType.Tanh,
                    )
                    d_t = gates.tile([128, 2, Bl], f32)
                    nc.vector.tensor_tensor(
                        out=d_t[:], in0=h32[:], in1=n_t[:], op=AluOpType.subtract
                    )
                    zd_t = gates.tile([128, 2, Bl], f32)
                    nc.vector.tensor_tensor(
                        out=zd_t[:], in0=rz_t[:, 2:4, :], in1=d_t[:], op=AluOpType.mult
                    )
                    nc.vector.tensor_tensor(
                        out=h32[:], in0=n_t[:], in1=zd_t[:], op=AluOpType.add
                    )
                    nc.vector.tensor_copy(out=hb[:], in_=h32[:])

            for c in (0, 1):
                nc.sync.dma_start(out=d_hT[c * 128:(c + 1) * 128, :], in_=h32[:, c, :])

    nc.finalize()
    return nc


def kernel(x, W_ih, W_hh, b_ih, b_hh, lengths, unsorted_indices):
    import os

    T, B, E = x.shape
    H = W_hh.shape[1]
    G3 = 3 * H
    Bl = B // N_CORES
    TB = T * Bl

    key = (T, Bl, E, H)
    if key not in _PROGRAM_CACHE:
        _PROGRAM_CACHE[key] = _build_program(T, Bl, E, H)
    nc = _PROGRAM_CACHE[key]

    x = np.asarray(x, np.float32)
    W_ih = np.asarray(W_ih, np.float32)
    W_hh = np.asarray(W_hh, np.float32)
    b_ih = np.asarray(b_ih, np.float32)
    b_hh = np.asarray(b_hh, np.float32)
    lengths = np.asarray(lengths)
    unsorted_indices = np.asarray(unsorted_indices)

    # shared (replicated) tensors
    wihT = np.ascontiguousarray(W_ih.T).astype(BF16)          # [E, 3H]
    whhT = np.ascontiguousarray(W_hh.T).astype(BF16)          # [H, 3H]
    bias_row = b_ih + b_hh
    bias_row[2 * H:] = b_ih[2 * H:]                           # n gate: b_ih only
    zind = np.zeros(G3, np.float32)
    zind[H:2 * H] = 1.0                                       # z-gate rows
    biasL = np.stack([bias_row, zind], 0).astype(BF16)        # [2, 3H]
    bhn = b_hh[2 * H:].reshape(2, 128).T.copy()               # [128, 2] fp32

    tvec = np.arange(T, dtype=np.int64)[:, None]
    in_maps = []
    for c in range(N_CORES):
        rows = slice(c * Bl, (c + 1) * Bl)
        xc = np.ascontiguousarray(x[:, rows, :].transpose(2, 0, 1)).reshape(E, TB)
        msk = tvec >= np.asarray(lengths[rows], np.int64)[None, :]  # [T, Bl] inactive
        bmr = np.stack(
            [np.ones(TB, np.float32), (msk.astype(np.float32) * 30.0).reshape(TB)], 0
        )
        in_maps.append(
            {
                "xT": xc.astype(BF16),
                "wihT": wihT,
                "whhT": whhT,
                "biasL": biasL,
                "bmr": bmr.astype(BF16),
                "bhn": bhn.astype(np.float32),
            }
        )

    trace = bool(int(os.environ.get("GRU_TRACE", "0")))
    res = run_bass_kernel_spmd(nc, in_maps, list(range(N_CORES)), trace=trace)
    global LAST_RESULT
    LAST_RESULT = res

    h = np.zeros((B, H), np.float32)
    for c in range(N_CORES):
        hT = np.asarray(res.results[c]["hT"], np.float32)  # [2*128, Bl]
        h[c * Bl:(c + 1) * Bl] = hT.T
    h = h[unsorted_indices]
    return h[None].astype(np.float32)


# revision 9
# speedup vs baseline: 1.1474x; 1.1474x over previous
"""Packed-sequence GRU (single layer) on 8 Trainium2 NeuronCores.

Data-parallel over batch (32 rows/core), with each core's rows further
split into G phase-shifted groups whose independent recurrence chains
overlap on the five engines (PE runs group B's matmuls while group A's
gate chain occupies ACT/DVE/GpSimd).

Device-side structure per group and 8-step block:
  - Gi = W_ih @ x^T is computed into PSUM (x is host-transposed/bf16, so
    no on-device transpose); biases, the packed-sequence length mask
    (z-gate saturation: +30 on inactive (t,b) makes z -> 1 so frozen rows
    keep h exactly like the reference masking), and b_hn are all folded
    in with K=2 matmuls.
  - The recurrent matmul gh = W_hh @ h accumulates straight onto the Gi
    PSUM regions for r/z (PE does the gi+gh add); the n gate keeps the
    gi / gh parts separate (GRU applies r only to the gh part).
  - Gates run in [feature-partition, batch-free] layout:
      rz   = sigmoid(PSUM)                (ScalarE)
      p    = r * (gh_n + b_hn)            (VectorE, b_hn pre-added in PSUM)
      q    = p + gi_n                     (VectorE)
      n    = tanh(q)                      (ScalarE)
      zh   = z * h                        (GpSimd, runs under tanh)
      m    = (z - 1) * n                  (VectorE fused scalar_tensor_tensor)
      h'   = zh - m                       (VectorE)  [= (1-z)n + zh]
      hb   = bf16(h')                     (GpSimd, feeds next matmul)
"""

import sys

for _p in ("/opt/trn_rl_repo", "/root/.axon_site/_ro/trn_rl_repo"):
    if _p not in sys.path:
        sys.path.insert(0, _p)

import numpy as np
import ml_dtypes

import concourse.bacc as bacc
import concourse.tile as tile
from concourse.tile import add_dep_helper
from concourse import mybir
from concourse.bass_utils import run_bass_kernel_spmd
from concourse.alu_op_type import AluOpType

BF16 = ml_dtypes.bfloat16
N_CORES = 8

_PROGRAM_CACHE = {}
LAST_RESULT = None  # stashed BassKernelResults for the test harness


def _build_program(T, Bl, E, H, G=2, S=8):
    assert E == 256 and H == 256 and Bl == 32 and T % S == 0
    Bg = Bl // G          # rows per group
    R = S * Bg            # columns per (jt, block) PSUM sub-region
    NB = T // S           # number of blocks
    G3 = 3 * H
    bf = mybir.dt.bfloat16
    f32 = mybir.dt.float32

    nc = bacc.Bacc(None, target_bir_lowering=False)
    d_xT = nc.declare_dram_parameter("xT", [E, T * Bl], bf, False)
    d_wih = nc.declare_dram_parameter("wihT", [E, G3], bf, False)
    d_whh = nc.declare_dram_parameter("whhT", [H, G3], bf, False)
    d_bias = nc.declare_dram_parameter("biasL", [2, G3], bf, False)
    d_bmr = nc.declare_dram_parameter("bmr", [2, T * Bl], bf, False)
    d_bhn = nc.declare_dram_parameter("bhnL", [2, 128], bf, False)
    d_cind = nc.declare_dram_parameter("cind", [2, 2 * R], bf, False)
    d_hT = nc.declare_dram_parameter("hT", [2 * 128, Bl], f32, True)

    with tile.TileContext(nc) as tc:
        with (
            tc.tile_pool(name="consts", bufs=1) as consts,
            tc.tile_pool(name="psum", bufs=2, space="PSUM") as psum,
            tc.tile_pool(name="gates", bufs=3) as gates,
        ):
            xsb = consts.tile([128, 2, T * Bl], bf)
            wih_sb = consts.tile([128, 2, G3], bf)
            whh_sb = consts.tile([128, 2, G3], bf)
            bias_sb = consts.tile([2, G3], bf)
            bmr_sb = consts.tile([2, T * Bl], bf)
            bhn_sb = consts.tile([2, 128], bf)
            cind_sb = consts.tile([2, 2 * R], bf)
            h32 = [consts.tile([128, 2, Bg], f32, name=f"h32_{g}") for g in range(G)]
            hb = [consts.tile([128, 2, Bg], bf, name=f"hb_{g}") for g in range(G)]

            for c in (0, 1):
                nc.sync.dma_start(out=xsb[:, c, :], in_=d_xT[c * 128:(c + 1) * 128, :])
                nc.sync.dma_start(out=wih_sb[:, c, :], in_=d_wih[c * 128:(c + 1) * 128, :])
                nc.sync.dma_start(out=whh_sb[:, c, :], in_=d_whh[c * 128:(c + 1) * 128, :])
            nc.sync.dma_start(out=bias_sb[:], in_=d_bias[:])
            nc.sync.dma_start(out=bmr_sb[:], in_=d_bmr[:])
            nc.sync.dma_start(out=bhn_sb[:], in_=d_bhn[:])
            nc.sync.dma_start(out=cind_sb[:], in_=d_cind[:])
            for g in range(G):
                nc.vector.memset(h32[g][:], 0.0)
                nc.vector.memset(hb[g][:], 0.0)

            # Per-(group, block) PSUM tile [128, 8*R] layout (free offsets):
            #   rz   : jt*R + s*Bg + b      jt in 0..3  (r_c0, r_c1, z_c0, z_c1)
            #   n_gi : 4R + c*R + s*Bg + b
            #   n_gh : 6R + s*2*Bg + c*Bg + b
            def fill_block(pb, g, blk):
                col0 = (g * T + blk * S) * Bg
                # start=True (clears the whole bank's has_written bits) only
                # on the first matmul of each 512-column PSUM bank.
                bank_clearers = {}
                for jt in range(6):
                    off = jt * R if jt < 4 else 4 * R + (jt - 4) * R
                    for kc in (0, 1):
                        clears = kc == 0 and off % 512 == 0
                        mm = nc.tensor.matmul(
                            pb[:, off:off + R],
                            lhsT=wih_sb[:, kc, jt * 128:(jt + 1) * 128],
                            rhs=xsb[:, kc, col0:col0 + R],
                            start=clears,
                            stop=False,
                        )
                        if clears:
                            bank_clearers[off // 512] = mm
                for jt in range(6):
                    off = jt * R if jt < 4 else 4 * R + (jt - 4) * R
                    nc.tensor.matmul(
                        pb[:, off:off + R],
                        lhsT=bias_sb[:, jt * 128:(jt + 1) * 128],
                        rhs=bmr_sb[:, col0:col0 + R],
                        start=False,
                        stop=False,
                    )
                # b_hn broadcast into the n_gh ring (c-indicator rhs).
                # With G==1 the n_gh ring is a whole bank and this is its
                # first write of the block: start=True clears stale
                # has_written bits. With G==2 the Gi jt4 matmul clears the
                # shared bank — and because this matmul's inputs are ready
                # long before the x-gated Gi matmuls, the scheduler would
                # hoist it ahead of that clear (wiping its has_written bits)
                # without an explicit ordering edge.
                pre_bank = (6 * R) // 512
                pre = nc.tensor.matmul(
                    pb[:, 6 * R:8 * R],
                    lhsT=bhn_sb[:],
                    rhs=cind_sb[:],
                    start=(6 * R) % 512 == 0,
                    stop=False,
                )
                if pre_bank in bank_clearers:
                    add_dep_helper(
                        pre.ins, bank_clearers[pre_bank].ins, sync=False,
                        reason="b_hn preload must follow the bank-clearing Gi matmul",
                    )

            def step(pb, g, s):
                for jt in range(4):
                    off = jt * R + s * Bg
                    for kc in (0, 1):
                        nc.tensor.matmul(
                            pb[:, off:off + Bg],
                            lhsT=whh_sb[:, kc, jt * 128:(jt + 1) * 128],
                            rhs=hb[g][:, kc, :],
                            start=False,
                            stop=(kc == 1),
                        )
                for jt in (4, 5):
                    off = 6 * R + s * 2 * Bg + (jt - 4) * Bg
                    for kc in (0, 1):
                        nc.tensor.matmul(
                            pb[:, off:off + Bg],
                            lhsT=whh_sb[:, kc, jt * 128:(jt + 1) * 128],
                            rhs=hb[g][:, kc, :],
                            start=False,
                            stop=(kc == 1),
                        )

            def gates_ops(pb, g, s):
                rz_view = pb[:, 0:4 * R].rearrange(
                    "p (jt s b) -> p jt s b", jt=4, s=S
                )[:, :, s, :]
                ngi_view = pb[:, 4 * R:6 * R].rearrange(
                    "p (c s b) -> p c s b", c=2, s=S
                )[:, :, s, :]
                ngh_view = pb[:, 6 * R + s * 2 * Bg: 6 * R + (s + 1) * 2 * Bg]

                rz_t = gates.tile([128, 4, Bg], f32, name=f"rz_{g}")
                nc.scalar.activation(
                    out=rz_t[:], in_=rz_view,
                    func=mybir.ActivationFunctionType.Sigmoid,
                )
                zh_t = gates.tile([128, 2, Bg], f32, name=f"zh_{g}")
                nc.gpsimd.tensor_tensor(
                    out=zh_t[:], in0=rz_t[:, 2:4, :], in1=h32[g][:], op=AluOpType.mult
                )
                p_t = gates.tile([128, 2, Bg], f32, name=f"p_{g}")
                nc.vector.tensor_tensor(
                    out=p_t[:], in0=ngh_view, in1=rz_t[:, 0:2, :], op=AluOpType.mult
                )
                q_t = gates.tile([128, 2, Bg], f32, name=f"q_{g}")
                nc.vector.tensor_tensor(
                    out=q_t[:], in0=p_t[:], in1=ngi_view, op=AluOpType.add
                )
                n_t = gates.tile([128, 2, Bg], f32, name=f"n_{g}")
                nc.scalar.activation(
                    out=n_t[:], in_=q_t[:],
                    func=mybir.ActivationFunctionType.Tanh,
                )
                m_t = gates.tile([128, 2, Bg], f32, name=f"m_{g}")
                nc.vector.scalar_tensor_tensor(
                    out=m_t[:], in0=rz_t[:, 2:4, :], scalar=1.0, in1=n_t[:],
                    op0=AluOpType.subtract, op1=AluOpType.mult,
                )
                nc.vector.tensor_tensor(
                    out=h32[g][:], in0=zh_t[:], in1=m_t[:], op=AluOpType.subtract
                )
                nc.gpsimd.tensor_copy(out=hb[g][:], in_=h32[g][:])

            for blk in range(NB):
                pbs = [
                    psum.tile([128, 8 * R], f32, name=f"pb_{g}") for g in range(G)
                ]
                for g in range(G):
                    fill_block(pbs[g], g, blk)
                for s in range(S):
                    for g in range(G):
                        step(pbs[g], g, s)
                    for g in range(G):
                        gates_ops(pbs[g], g, s)

            for g in range(G):
                for c in (0, 1):
                    nc.sync.dma_start(
                        out=d_hT[c * 128:(c + 1) * 128, g * Bg:(g + 1) * Bg],
                        in_=h32[g][:, c, :],
                    )

    nc.finalize()
    return nc


def kernel(x, W_ih, W_hh, b_ih, b_hh, lengths, unsorted_indices):
    import os

    T, B, E = x.shape
    H = W_hh.shape[1]
    G3 = 3 * H
    Bl = B // N_CORES
    G = int(os.environ.get("GRU_GROUPS", "2"))
    S = 8
    Bg = Bl // G
    R = S * Bg

    key = (T, Bl, E, H, G)
    if key not in _PROGRAM_CACHE:
        _PROGRAM_CACHE[key] = _build_program(T, Bl, E, H, G=G, S=S)
    nc = _PROGRAM_CACHE[key]

    x = np.asarray(x, np.float32)
    W_ih = np.asarray(W_ih, np.float32)
    W_hh = np.asarray(W_hh, np.float32)
    b_ih = np.asarray(b_ih, np.float32)
    b_hh = np.asarray(b_hh, np.float32)
    lengths = np.asarray(lengths)
    unsorted_indices = np.asarray(unsorted_indices)

    wihT = np.ascontiguousarray(W_ih.T).astype(BF16)          # [E, 3H]
    whhT = np.ascontiguousarray(W_hh.T).astype(BF16)          # [H, 3H]
    bias_row = b_ih + b_hh
    bias_row[2 * H:] = b_ih[2 * H:]                           # n gate: b_ih only
    zind = np.zeros(G3, np.float32)
    zind[H:2 * H] = 1.0                                       # z-gate rows
    biasL = np.stack([bias_row, zind], 0).astype(BF16)        # [2, 3H]
    bhnL = b_hh[2 * H:].reshape(2, 128).astype(BF16)          # [2, 128]
    # c-indicator for broadcasting b_hn into the n_gh ring: [2, (s, c, b)]
    cind = np.zeros((2, S, 2, Bg), np.float32)
    cind[0, :, 0, :] = 1.0
    cind[1, :, 1, :] = 1.0
    cind = cind.reshape(2, 2 * R).astype(BF16)

    tvec = np.arange(T, dtype=np.int64)[:, None]
    in_maps = []
    for c in range(N_CORES):
        rows = slice(c * Bl, (c + 1) * Bl)
        # pack as [E, (g, t, b)] so each group's block columns are contiguous
        xc = x[:, rows, :].transpose(2, 0, 1).reshape(E, T, G, Bg)
        xc = np.ascontiguousarray(xc.transpose(0, 2, 1, 3)).reshape(E, T * Bl)
        msk = (tvec >= np.asarray(lengths[rows], np.int64)[None, :]).astype(np.float32)
        msk = msk.reshape(T, G, Bg).transpose(1, 0, 2).reshape(T * Bl)
        bmr = np.stack([np.ones(T * Bl, np.float32), msk * 30.0], 0)
        in_maps.append(
            {
                "xT": xc.astype(BF16),
                "wihT": wihT,
                "whhT": whhT,
                "biasL": biasL,
                "bmr": bmr.astype(BF16),
                "bhnL": bhnL,
                "cind": cind,
            }
        )

    trace = bool(int(os.environ.get("GRU_TRACE", "0")))
    res = run_bass_kernel_spmd(nc, in_maps, list(range(N_CORES)), trace=trace)
    global LAST_RESULT
    LAST_RESULT = res

    h = np.zeros((B, H), np.float32)
    for c in range(N_CORES):
        hT = np.asarray(res.results[c]["hT"], np.float32)  # [2*128, Bl]
        h[c * Bl:(c + 1) * Bl] = hT.T
    h = h[unsorted_indices]
    return h[None].astype(np.float32)


# revision 10
# speedup vs baseline: 1.2891x; 1.1235x over previous
"""Packed-sequence GRU (single layer) on 8 Trainium2 NeuronCores.

Data-parallel over batch (32 rows/core), with each core's rows further
split into G phase-shifted groups whose independent recurrence chains
overlap on the five engines (PE runs group B's matmuls while group A's
gate chain occupies ACT/DVE/GpSimd).

Device-side structure per group and 8-step block:
  - Gi = W_ih @ x^T is computed into PSUM (x is host-transposed/bf16, so
    no on-device transpose); biases, the packed-sequence length mask
    (z-gate saturation: +30 on inactive (t,b) makes z -> 1 so frozen rows
    keep h exactly like the reference masking), and b_hn are all folded
    in with K=2 matmuls.
  - The recurrent matmul gh = W_hh @ h accumulates straight onto the Gi
    PSUM regions for r/z (PE does the gi+gh add); the n gate keeps the
    gi / gh parts separate (GRU applies r only to the gh part).
  - Gates run in [feature-partition, batch-free] layout:
      rz   = sigmoid(PSUM)                (ScalarE)
      p    = r * (gh_n + b_hn)            (VectorE, b_hn pre-added in PSUM)
      q    = p + gi_n                     (VectorE)
      n    = tanh(q)                      (ScalarE)
      zh   = z * h                        (GpSimd, runs under tanh)
      m    = (z - 1) * n                  (VectorE fused scalar_tensor_tensor)
      h'   = zh - m                       (VectorE)  [= (1-z)n + zh]
      hb   = bf16(h')                     (GpSimd, feeds next matmul)
"""

import sys

for _p in ("/opt/trn_rl_repo", "/root/.axon_site/_ro/trn_rl_repo"):
    if _p not in sys.path:
        sys.path.insert(0, _p)

import numpy as np
import ml_dtypes

import concourse.bacc as bacc
import concourse.tile as tile
from concourse.tile import add_dep_helper
from concourse import mybir
from concourse.bass_utils import run_bass_kernel_spmd
from concourse.alu_op_type import AluOpType

BF16 = ml_dtypes.bfloat16
N_CORES = 8

_PROGRAM_CACHE = {}
LAST_RESULT = None  # stashed BassKernelResults for the test harness


def _build_program(T, Bl, E, H, G=2, S=8):
    assert E == 256 and H == 256 and Bl == 32 and T % S == 0
    Bg = Bl // G          # rows per group
    R = S * Bg            # columns per (jt, block) PSUM sub-region
    NB = T // S           # number of blocks
    G3 = 3 * H
    bf = mybir.dt.bfloat16
    f32 = mybir.dt.float32

    nc = bacc.Bacc(None, target_bir_lowering=False)
    d_xT = nc.declare_dram_parameter("xT", [E, T * Bl], bf, False)
    d_wih = nc.declare_dram_parameter("wihT", [E, G3], bf, False)
    d_whh = nc.declare_dram_parameter("whhT", [H, G3], bf, False)
    d_bias = nc.declare_dram_parameter("biasL", [2, G3], bf, False)
    d_bmr = nc.declare_dram_parameter("bmr", [2, T * Bl], bf, False)
    d_bhn = nc.declare_dram_parameter("bhnL", [2, 128], bf, False)
    d_cind = nc.declare_dram_parameter("cind", [2, 2 * R], bf, False)
    d_hT = nc.declare_dram_parameter("hT", [2 * 128, Bl], bf, True)

    with tile.TileContext(nc) as tc:
        with (
            tc.tile_pool(name="consts", bufs=1) as consts,
            tc.tile_pool(name="psum", bufs=2, space="PSUM") as psum,
            tc.tile_pool(name="gates", bufs=3) as gates,
        ):
            xsb = consts.tile([128, 2, T * Bl], bf)
            wih_sb = consts.tile([128, 2, G3], bf)
            whh_sb = consts.tile([128, 2, G3], bf)
            bias_sb = consts.tile([2, G3], bf)
            bmr_sb = consts.tile([2, T * Bl], bf)
            bhn_sb = consts.tile([2, 128], bf)
            cind_sb = consts.tile([2, 2 * R], bf)
            hb = [consts.tile([128, 2, Bg], bf, name=f"hb_{g}") for g in range(G)]

            for c in (0, 1):
                nc.sync.dma_start(out=xsb[:, c, :], in_=d_xT[c * 128:(c + 1) * 128, :])
                nc.sync.dma_start(out=wih_sb[:, c, :], in_=d_wih[c * 128:(c + 1) * 128, :])
                nc.sync.dma_start(out=whh_sb[:, c, :], in_=d_whh[c * 128:(c + 1) * 128, :])
            nc.sync.dma_start(out=bias_sb[:], in_=d_bias[:])
            nc.sync.dma_start(out=bmr_sb[:], in_=d_bmr[:])
            nc.sync.dma_start(out=bhn_sb[:], in_=d_bhn[:])
            nc.sync.dma_start(out=cind_sb[:], in_=d_cind[:])
            for g in range(G):
                nc.vector.memset(hb[g][:], 0.0)

            # Per-(group, block) PSUM tile [128, 8*R] layout (free offsets):
            #   rz   : jt*R + s*Bg + b      jt in 0..3  (r_c0, r_c1, z_c0, z_c1)
            #   n_gi : 4R + c*R + s*Bg + b
            #   n_gh : 6R + s*2*Bg + c*Bg + b
            def fill_block(pb, g, blk):
                col0 = (g * T + blk * S) * Bg
                # start=True (clears the whole bank's has_written bits) only
                # on the first matmul of each 512-column PSUM bank.
                bank_clearers = {}
                for jt in range(6):
                    off = jt * R if jt < 4 else 4 * R + (jt - 4) * R
                    for kc in (0, 1):
                        clears = kc == 0 and off % 512 == 0
                        mm = nc.tensor.matmul(
                            pb[:, off:off + R],
                            lhsT=wih_sb[:, kc, jt * 128:(jt + 1) * 128],
                            rhs=xsb[:, kc, col0:col0 + R],
                            start=clears,
                            stop=False,
                        )
                        if clears:
                            bank_clearers[off // 512] = mm
                for jt in range(6):
                    off = jt * R if jt < 4 else 4 * R + (jt - 4) * R
                    nc.tensor.matmul(
                        pb[:, off:off + R],
                        lhsT=bias_sb[:, jt * 128:(jt + 1) * 128],
                        rhs=bmr_sb[:, col0:col0 + R],
                        start=False,
                        stop=False,
                    )
                # b_hn broadcast into the n_gh ring (c-indicator rhs).
                # With G==1 the n_gh ring is a whole bank and this is its
                # first write of the block: start=True clears stale
                # has_written bits. With G==2 the Gi jt4 matmul clears the
                # shared bank — and because this matmul's inputs are ready
                # long before the x-gated Gi matmuls, the scheduler would
                # hoist it ahead of that clear (wiping its has_written bits)
                # without an explicit ordering edge.
                pre_bank = (6 * R) // 512
                pre = nc.tensor.matmul(
                    pb[:, 6 * R:8 * R],
                    lhsT=bhn_sb[:],
                    rhs=cind_sb[:],
                    start=(6 * R) % 512 == 0,
                    stop=False,
                )
                if pre_bank in bank_clearers:
                    add_dep_helper(
                        pre.ins, bank_clearers[pre_bank].ins, sync=False,
                        reason="b_hn preload must follow the bank-clearing Gi matmul",
                    )

            def step(pb, g, s):
                for jt in range(4):
                    off = jt * R + s * Bg
                    for kc in (0, 1):
                        nc.tensor.matmul(
                            pb[:, off:off + Bg],
                            lhsT=whh_sb[:, kc, jt * 128:(jt + 1) * 128],
                            rhs=hb[g][:, kc, :],
                            start=False,
                            stop=(kc == 1),
                        )
                for jt in (4, 5):
                    off = 6 * R + s * 2 * Bg + (jt - 4) * Bg
                    for kc in (0, 1):
                        nc.tensor.matmul(
                            pb[:, off:off + Bg],
                            lhsT=whh_sb[:, kc, jt * 128:(jt + 1) * 128],
                            rhs=hb[g][:, kc, :],
                            start=False,
                            stop=(kc == 1),
                        )

            def gates_ops(pb, g, s):
                rz_view = pb[:, 0:4 * R].rearrange(
                    "p (jt s b) -> p jt s b", jt=4, s=S
                )[:, :, s, :]
                ngi_view = pb[:, 4 * R:6 * R].rearrange(
                    "p (c s b) -> p c s b", c=2, s=S
                )[:, :, s, :]
                ngh_view = pb[:, 6 * R + s * 2 * Bg: 6 * R + (s + 1) * 2 * Bg]

                rz_t = gates.tile([128, 4, Bg], f32, name=f"rz_{g}")
                nc.scalar.activation(
                    out=rz_t[:], in_=rz_view,
                    func=mybir.ActivationFunctionType.Sigmoid,
                )
                zh_t = gates.tile([128, 2, Bg], f32, name=f"zh_{g}")
                nc.gpsimd.tensor_tensor(
                    out=zh_t[:], in0=rz_t[:, 2:4, :], in1=hb[g][:], op=AluOpType.mult
                )
                p_t = gates.tile([128, 2, Bg], f32, name=f"p_{g}")
                nc.vector.tensor_tensor(
                    out=p_t[:], in0=ngh_view, in1=rz_t[:, 0:2, :], op=AluOpType.mult
                )
                q_t = gates.tile([128, 2, Bg], f32, name=f"q_{g}")
                nc.vector.tensor_tensor(
                    out=q_t[:], in0=p_t[:], in1=ngi_view, op=AluOpType.add
                )
                n_t = gates.tile([128, 2, Bg], f32, name=f"n_{g}")
                nc.scalar.activation(
                    out=n_t[:], in_=q_t[:],
                    func=mybir.ActivationFunctionType.Tanh,
                )
                m_t = gates.tile([128, 2, Bg], f32, name=f"m_{g}")
                nc.vector.scalar_tensor_tensor(
                    out=m_t[:], in0=rz_t[:, 2:4, :], scalar=1.0, in1=n_t[:],
                    op0=AluOpType.subtract, op1=AluOpType.mult,
                )
                nc.vector.tensor_tensor(
                    out=hb[g][:], in0=zh_t[:], in1=m_t[:], op=AluOpType.subtract
                )

            for blk in range(NB):
                pbs = [
                    psum.tile([128, 8 * R], f32, name=f"pb_{g}") for g in range(G)
                ]
                for g in range(G):
                    fill_block(pbs[g], g, blk)
                for s in range(S):
                    for g in range(G):
                        step(pbs[g], g, s)
                    for g in range(G):
                        gates_ops(pbs[g], g, s)

            for g in range(G):
                for c in (0, 1):
                    nc.sync.dma_start(
                        out=d_hT[c * 128:(c + 1) * 128, g * Bg:(g + 1) * Bg],
                        in_=hb[g][:, c, :],
                    )

    nc.finalize()
    return nc


def kernel(x, W_ih, W_hh, b_ih, b_hh, lengths, unsorted_indices):
    import os

    T, B, E = x.shape
    H = W_hh.shape[1]
    G3 = 3 * H
    Bl = B // N_CORES
    G = int(os.environ.get("GRU_GROUPS", "2"))
    S = 8
    Bg = Bl // G
    R = S * Bg

    key = (T, Bl, E, H, G)
    if key not in _PROGRAM_CACHE:
        _PROGRAM_CACHE[key] = _build_program(T, Bl, E, H, G=G, S=S)
    nc = _PROGRAM_CACHE[key]

    x = np.asarray(x, np.float32)
    W_ih = np.asarray(W_ih, np.float32)
    W_hh = np.asarray(W_hh, np.float32)
    b_ih = np.asarray(b_ih, np.float32)
    b_hh = np.asarray(b_hh, np.float32)
    lengths = np.asarray(lengths)
    unsorted_indices = np.asarray(unsorted_indices)

    wihT = np.ascontiguousarray(W_ih.T).astype(BF16)          # [E, 3H]
    whhT = np.ascontiguousarray(W_hh.T).astype(BF16)          # [H, 3H]
    bias_row = b_ih + b_hh
    bias_row[2 * H:] = b_ih[2 * H:]                           # n gate: b_ih only
    zind = np.zeros(G3, np.float32)
    zind[H:2 * H] = 1.0                                       # z-gate rows
    biasL = np.stack([bias_row, zind], 0).astype(BF16)        # [2, 3H]
    bhnL = b_hh[2 * H:].reshape(2, 128).astype(BF16)          # [2, 128]
    # c-indicator for broadcasting b_hn into the n_gh ring: [2, (s, c, b)]
    cind = np.zeros((2, S, 2, Bg), np.float32)
    cind[0, :, 0, :] = 1.0
    cind[1, :, 1, :] = 1.0
    cind = cind.reshape(2, 2 * R).astype(BF16)

    tvec = np.arange(T, dtype=np.int64)[:, None]
    in_maps = []
    for c in range(N_CORES):
        rows = slice(c * Bl, (c + 1) * Bl)
        # pack as [E, (g, t, b)] so each group's block columns are contiguous
        xc = x[:, rows, :].transpose(2, 0, 1).reshape(E, T, G, Bg)
        xc = np.ascontiguousarray(xc.transpose(0, 2, 1, 3)).reshape(E, T * Bl)
        msk = (tvec >= np.asarray(lengths[rows], np.int64)[None, :]).astype(np.float32)
        msk = msk.reshape(T, G, Bg).transpose(1, 0, 2).reshape(T * Bl)
        bmr = np.stack([np.ones(T * Bl, np.float32), msk * 30.0], 0)
        in_maps.append(
            {
                "xT": xc.astype(BF16),
                "wihT": wihT,
                "whhT": whhT,
                "biasL": biasL,
                "bmr": bmr.astype(BF16),
                "bhnL": bhnL,
                "cind": cind,
            }
        )

    trace = bool(int(os.environ.get("GRU_TRACE", "0")))
    res = run_bass_kernel_spmd(nc, in_maps, list(range(N_CORES)), trace=trace)
    global LAST_RESULT
    LAST_RESULT = res

    h = np.zeros((B, H), np.float32)
    for c in range(N_CORES):
        hT = np.asarray(res.results[c]["hT"], np.float32)  # [2*128, Bl]
        h[c * Bl:(c + 1) * Bl] = hT.T
    h = h[unsorted_indices]
    return h[None].astype(np.float32)
